# revision 5
# baseline (speedup 1.0000x reference)
"""Bahdanau attention Trainium2 kernel.

Problem: B=8, T=256, S=256, H=512 (fp32 I/O).
  Ws_q = q @ W_s.T ; Wh_e = e @ W_h.T
  energies[b,t,s] = v . tanh(Ws_q[b,t,:] + Wh_e[b,s,:])   (masked s >= len_b)
  attn = softmax_s(energies); ctx = attn @ e
  out = tanh(concat([ctx, q]) @ W_out.T)

Sharding: sequence-parallel over T — core c handles t in [c*32, (c+1)*32)
for ALL batches. This balances the src_lengths sparsity across cores
(each core's dominant tanh work is 32 * sum_b(len_b) * H instead of
256 * len_b * H for a single batch).

Per-core dataflow (bf16 compute, fp32 accumulation):
  PE   : Ws_q^T [o,t] and Wh_e^T [o,s] projections (o on partitions)
  DVE  : X[o, t, s] = Wh_e^T[o,s] + Ws_q^T[o,t]  (tensor_scalar_add, 4x bf16)
  ACT  : tanh(X) in place, one instruction per (b, o-chunk)
  PE   : energies[t,s] = sum_o v_o X[o,t,s] — M=1 matmuls col-tiled 4-wide
  DMA  : gather PSUM rows {0,32,64,96} -> energies [32t, s]
  DVE/ACT: masked softmax (exp with accum_out for the row sums)
  DMA  : xbar-transpose of weights [32,s] -> [s,32]
  PE   : ctx^T[h,t] = enc^T @ w^T ; out[t,o] = tanh(comb^T.T @ W_out^T)
"""

import functools
import math

import ml_dtypes
import numpy as np

B, T, S, H = 8, 256, 256, 512
NCORES = 8
TC = T // NCORES  # 32 target positions per core
KC = H // 128     # 4 contraction chunks
OC = H // 128     # 4 output-feature chunks

_BF16 = ml_dtypes.bfloat16


def _ceil4(x: int) -> int:
    return max(4, (x + 3) // 4 * 4)


@functools.lru_cache(maxsize=8)
def _build(lens: tuple):
    """Build + compile the per-core Bass program with per-batch s-extents
    baked in. Same program runs on all 8 cores (inputs differ)."""
    import concourse.mybir as mybir
    import concourse.tile as tile
    from concourse import bacc

    f32 = mybir.dt.float32
    bf16 = mybir.dt.bfloat16
    AF = mybir.ActivationFunctionType
    AX = mybir.AxisListType

    Ls = [_ceil4(l) for l in lens]

    nc = bacc.Bacc("TRN2", target_bir_lowering=False, debug=False)

    # All inputs are host-pre-arranged into SBUF layout [128, free].
    qt_d = nc.dram_tensor("qt", [128, KC, B, TC], bf16, kind="ExternalInput")
    encT_d = nc.dram_tensor("encT", [B, 128, KC, S], bf16, kind="ExternalInput")
    enc_d = nc.dram_tensor("enc", [B, 128, S // 128, H], bf16, kind="ExternalInput")
    wst_d = nc.dram_tensor("wst", [128, KC, H], bf16, kind="ExternalInput")
    wht_d = nc.dram_tensor("wht", [128, KC, H], bf16, kind="ExternalInput")
    v_d = nc.dram_tensor("v", [128, KC], bf16, kind="ExternalInput")
    wot_d = nc.dram_tensor("wot", [128, 2 * KC, H], bf16, kind="ExternalInput")
    out_d = nc.dram_tensor("out", [B, TC, H], f32, kind="ExternalOutput")

    with tile.TileContext(nc) as tc:
        with (
            tc.tile_pool(name="const", bufs=1) as constp,
            tc.tile_pool(name="enc", bufs=3) as encp,
            tc.tile_pool(name="es", bufs=2) as esp,
            tc.tile_pool(name="x", bufs=2) as xp,
            tc.tile_pool(name="sm", bufs=3) as smp,
            tc.tile_pool(name="outs", bufs=3) as outp,
            tc.tile_pool(name="psA", bufs=2, space="PSUM") as psA,
            tc.tile_pool(name="psV", bufs=2, space="PSUM") as psV,
            tc.tile_pool(name="psC", bufs=1, space="PSUM") as psC,
            tc.tile_pool(name="psO", bufs=1, space="PSUM") as psO,
        ):
            # ---- persistent weights/activations ----
            wst = constp.tile([128, KC, H], bf16)
            nc.sync.dma_start(wst[:], wst_d[:])
            wht = constp.tile([128, KC, H], bf16)
            nc.sync.dma_start(wht[:], wht_d[:])
            v_sb = constp.tile([128, KC], bf16)
            nc.sync.dma_start(v_sb[:], v_d[:])
            wot = constp.tile([128, 2 * KC, H], bf16)
            nc.sync.dma_start(wot[:], wot_d[:])
            qt_sb = constp.tile([128, KC, B, TC], bf16)
            nc.sync.dma_start(qt_sb[:], qt_d[:])

            # ---- Ws_q^T for all (b, t): qs[o-part, oc, b, t] (fp32 scalars) ----
            qs_sb = constp.tile([128, OC, B, TC], f32)
            for oc in range(OC):
                ps = psA.tile([128, B * TC], f32, tag="proj")
                for kc in range(KC):
                    nc.tensor.matmul(
                        ps[:],
                        wst[:, kc, oc * 128 : (oc + 1) * 128],
                        qt_sb[:, kc, :, :],
                        start=(kc == 0),
                        stop=(kc == KC - 1),
                    )
                nc.vector.tensor_copy(
                    qs_sb[:, oc, :, :], ps.rearrange("p (b t) -> p b t", b=B)
                )

            for b in range(B):
                L = Ls[b]
                ln = min(int(lens[b]), S)
                SC = (L + 127) // 128  # 1 or 2 s-chunks of 128 for ctx matmul
                L128 = SC * 128

                # ---- load encoder (both layouts), full-S tiles for clean DMA ----
                encT_b = encp.tile([128, KC, S], bf16, tag="encT")
                nc.sync.dma_start(encT_b[:], encT_d[b])
                enc_b = encp.tile([128, S // 128, H], bf16, tag="enc")
                nc.sync.dma_start(enc_b[:], enc_d[b])

                # ---- Wh_e^T: es[o-part, s] per oc ----
                es = []
                for oc in range(OC):
                    ps = psA.tile([128, L], f32, tag="proj")
                    for kc in range(KC):
                        nc.tensor.matmul(
                            ps[:],
                            wht[:, kc, oc * 128 : (oc + 1) * 128],
                            encT_b[:, kc, :L],
                            start=(kc == 0),
                            stop=(kc == KC - 1),
                        )
                    e = esp.tile([128, L], bf16, tag=f"es{oc}")
                    nc.vector.tensor_copy(e[:], ps[:])
                    es.append(e)

                # ---- X[o, t, s] = es[o, s] + qs[o, t]; tanh in place ----
                X = []
                for oc in range(OC):
                    x = xp.tile([128, TC, L], bf16, tag=f"x{oc}")
                    for t in range(TC):
                        nc.vector.tensor_scalar_add(
                            x[:, t, :], es[oc][:], qs_sb[:, oc, b, t : t + 1]
                        )
                    nc.scalar.activation(x[:], x[:], AF.Tanh)
                    X.append(x)

                # ---- energies[t, s] = sum_o v_o X[o, t, s] ----
                # M=1 matmuls; 16 target positions packed into one PSUM tile:
                # 4 col groups (partitions {0,32,64,96}) x 4 bank slots.
                # Evacuate with one wide DVE copy, then SBUF->SBUF DMA gather.
                energ = smp.tile([32, L], f32, tag="energ")
                for h in range(TC // 16):
                    psq = psV.tile([128, 4, 512 // 2], f32, tag="vdot")
                    # n outer: accumulation groups sharing a (partition, bank)
                    # zero-region must not interleave — start=True marks the
                    # whole 2KB bank-row pending-zero. Col groups (distinct
                    # partitions) may interleave freely.
                    for n in range(4):
                        for oc in range(OC):
                            for j in range(4):
                                t = h * 16 + 4 * n + j
                                nc.tensor.matmul(
                                    psq[32 * j : 32 * j + 1, n, :L],
                                    v_sb[:, oc : oc + 1],
                                    X[oc][:, t, :],
                                    start=(oc == 0),
                                    stop=(oc == OC - 1),
                                    tile_position=(0, 32 * j),
                                )
                    vscr = smp.tile([128, 4, L], f32, tag="vscr")
                    nc.vector.tensor_copy(vscr[:], psq[:, :, :L])
                    # one DMA per bank slot: src partitions {0,32,64,96} ->
                    # 4 contiguous energ rows (free->partition mapping in a
                    # single DMA is not supported, partition->partition is)
                    vsr = vscr.rearrange("(g r) n f -> g r n f", r=32)
                    for n in range(4):
                        nc.sync.dma_start(
                            energ[h * 16 + 4 * n : h * 16 + 4 * n + 4, :],
                            vsr[:, 0, n, :],
                        )

                # ---- softmax over s (energies are bounded: |e| <= sum|v|
                # ~ 20, so raw exp is safe in fp32/bf16 — skip max-subtract) ----
                if L > ln:
                    nc.vector.memset(energ[:, ln:L], -1e9)
                w_sb = smp.tile([32, L128], bf16, tag="w")
                if L128 > L:
                    nc.vector.memset(w_sb[:, L:], 0.0)
                sm = smp.tile([32, 1], f32, tag="sm")
                nc.scalar.activation(
                    w_sb[:, :L], energ[:], AF.Exp, accum_out=sm[:]
                )
                rs = smp.tile([32, 1], f32, tag="rs")
                nc.vector.reciprocal(rs[:], sm[:])
                nc.vector.tensor_scalar_mul(w_sb[:, :L], w_sb[:, :L], rs[:])

                # ---- w^T via DMA xbar transpose: [32, L128] -> [L128, 32] ----
                wT = smp.tile([128, SC, TC], bf16, tag="wT")
                for sc in range(SC):
                    nc.sync.dma_start_transpose(
                        wT[:, sc, :], w_sb[:, sc * 128 : (sc + 1) * 128]
                    )

                # ---- ctx^T[h, t] = sum_s enc[s, h] * w[t, s] ----
                ctxT = outp.tile([128, OC, TC], bf16, tag="ctxT")
                for oc in range(OC):
                    psc = psC.tile([128, TC], f32, tag="ctx")
                    for sc in range(SC):
                        nc.tensor.matmul(
                            psc[:],
                            enc_b[:, sc, oc * 128 : (oc + 1) * 128],
                            wT[:, sc, :],
                            start=(sc == 0),
                            stop=(sc == SC - 1),
                        )
                    nc.vector.tensor_copy(ctxT[:, oc, :], psc[:])

                # ---- out[t, o] = tanh(sum_k comb^T[k, t] * W_out[o, k]) ----
                pso = psO.tile([32, H], f32, tag="outp")
                for kc in range(2 * KC):
                    lhsT = (
                        ctxT[:, kc, :]
                        if kc < OC
                        else qt_sb[:, kc - OC, b, :]
                    )
                    nc.tensor.matmul(
                        pso[:],
                        lhsT,
                        wot[:, kc, :],
                        start=(kc == 0),
                        stop=(kc == 2 * KC - 1),
                    )
                ob = outp.tile([32, H], f32, tag="ob")
                nc.scalar.activation(ob[:], pso[:], AF.Tanh)
                nc.sync.dma_start(out_d[b], ob[:])

    nc.compile()
    return nc


def _prep_inputs(query, encoder_outputs, src_lengths, W_s, W_h, v, W_out):
    """Host-side: cast to bf16 and pre-arrange into SBUF layouts."""
    q = np.asarray(query, np.float32)
    e = np.asarray(encoder_outputs, np.float32)

    # [128, KC, B, TC] per core: qt[p, kc, b, t] = q[b, c*TC+t, kc*128+p]
    # build once for full T then slice per core.
    qt_full = np.transpose(
        q.reshape(B, T, KC, 128), (2, 3, 0, 1)
    )  # [KC, 128, B, T]
    qt_full = np.ascontiguousarray(np.swapaxes(qt_full, 0, 1)).astype(_BF16)
    # -> [128, KC, B, T]

    # encT[b, p, kc, s] = e[b, s, kc*128+p]
    encT = np.ascontiguousarray(
        np.transpose(e.reshape(B, S, KC, 128), (0, 3, 2, 1))
    ).astype(_BF16)
    # enc[b, p, sc, h] = e[b, sc*128+p, h]
    enc = np.ascontiguousarray(
        np.transpose(e.reshape(B, S // 128, 128, H), (0, 2, 1, 3))
    ).astype(_BF16)

    # wst[p, kc, o] = W_s[o, kc*128+p]
    wst = np.ascontiguousarray(
        np.transpose(np.asarray(W_s, np.float32).reshape(H, KC, 128), (2, 1, 0))
    ).astype(_BF16)
    wht = np.ascontiguousarray(
        np.transpose(np.asarray(W_h, np.float32).reshape(H, KC, 128), (2, 1, 0))
    ).astype(_BF16)
    # v[p, kc] = v[kc*128+p]
    v_pre = np.ascontiguousarray(
        np.asarray(v, np.float32).reshape(KC, 128).T
    ).astype(_BF16)
    # wot[p, kc, o] = W_out[o, kc*128+p]   (k = 2H contraction)
    wot = np.ascontiguousarray(
        np.transpose(np.asarray(W_out, np.float32).reshape(H, 2 * KC, 128), (2, 1, 0))
    ).astype(_BF16)

    lens = tuple(int(x) for x in np.asarray(src_lengths).reshape(-1))
    return qt_full, encT, enc, wst, wht, v_pre, wot, lens


def kernel(query, encoder_outputs, src_lengths, W_s, W_h, v, W_out):
    from concourse import bass_utils

    qt_full, encT, enc, wst, wht, v_pre, wot, lens = _prep_inputs(
        query, encoder_outputs, src_lengths, W_s, W_h, v, W_out
    )
    nc = _build(lens)

    in_maps = []
    for c in range(NCORES):
        qt_c = np.ascontiguousarray(
            qt_full[:, :, :, c * TC : (c + 1) * TC]
        )
        in_maps.append(
            {
                "qt": qt_c,
                "encT": encT,
                "enc": enc,
                "wst": wst,
                "wht": wht,
                "v": v_pre,
                "wot": wot,
            }
        )

    res = bass_utils.run_bass_kernel_spmd(nc, in_maps, core_ids=list(range(NCORES)))

    out = np.empty((B, T, H), np.float32)
    for c in range(NCORES):
        out[:, c * TC : (c + 1) * TC, :] = res.results[c]["out"]
    return out


# revision 18
# speedup vs baseline: 2938.5105x; 2938.5105x over previous
"""Bahdanau attention Trainium2 kernel.

Problem: B=8, T=256, S=256, H=512 (fp32 I/O).
  Ws_q = q @ W_s.T ; Wh_e = e @ W_h.T
  energies[b,t,s] = v . tanh(Ws_q[b,t,:] + Wh_e[b,s,:])   (masked s >= len_b)
  attn = softmax_s(energies); ctx = attn @ e
  out = tanh(concat([ctx, q]) @ W_out.T)

Sharding: sequence-parallel over T — core c handles t in [c*32, (c+1)*32)
for ALL batches. This balances the src_lengths sparsity across cores
(each core's dominant tanh work is 32 * sum_b(len_b) * H instead of
256 * len_b * H for a single batch).

Per-core dataflow (bf16 compute, fp32 accumulation):
  PE   : Ws_q^T [o,t] and Wh_e^T [o,s] projections (o on partitions)
  DVE  : X[o, t, s] = Wh_e^T[o,s] + Ws_q^T[o,t]  (tensor_scalar_add, 4x bf16)
  ACT  : tanh(X) in place, one instruction per (b, o-chunk)
  PE   : energies[t,s] = sum_o v_o X[o,t,s] — M=1 matmuls col-tiled 4-wide
  DMA  : gather PSUM rows {0,32,64,96} -> energies [32t, s]
  DVE/ACT: masked softmax (exp with accum_out for the row sums)
  DMA  : xbar-transpose of weights [32,s] -> [s,32]
  PE   : ctx^T[h,t] = enc^T @ w^T ; out[t,o] = tanh(comb^T.T @ W_out^T)
"""

import functools
import math

import ml_dtypes
import numpy as np

B, T, S, H = 8, 256, 256, 512
NCORES = 8
TC = T // NCORES  # 32 target positions per core
KC = H // 128     # 4 contraction chunks
OC = H // 128     # 4 output-feature chunks

_BF16 = ml_dtypes.bfloat16


def _ceil4(x: int) -> int:
    return max(4, (x + 3) // 4 * 4)


@functools.lru_cache(maxsize=8)
def _build(lens: tuple, loop_n: int | None = None, stages: int = 3):
    """Build + compile the per-core Bass program with per-batch s-extents
    baked in. Same program runs on all 8 cores (inputs differ)."""
    import concourse.mybir as mybir
    import concourse.tile as tile
    from concourse import bacc

    f32 = mybir.dt.float32
    bf16 = mybir.dt.bfloat16
    AF = mybir.ActivationFunctionType
    AX = mybir.AxisListType

    Ls = [_ceil4(l) for l in lens]

    nc = bacc.Bacc("TRN2", target_bir_lowering=False, debug=False)

    # All inputs are host-pre-arranged into SBUF layout [128, free].
    qt_d = nc.dram_tensor("qt", [128, KC, B, TC], bf16, kind="ExternalInput")
    encT_d = nc.dram_tensor("encT", [B, 128, KC, S], bf16, kind="ExternalInput")
    enc_d = nc.dram_tensor("enc", [B, 128, S // 128, H], bf16, kind="ExternalInput")
    wst_d = nc.dram_tensor("wst", [128, KC, H], bf16, kind="ExternalInput")
    wht_d = nc.dram_tensor("wht", [128, KC, H], bf16, kind="ExternalInput")
    v_d = nc.dram_tensor("v", [128, KC], bf16, kind="ExternalInput")
    wot_d = nc.dram_tensor("wot", [128, 2 * KC, H], bf16, kind="ExternalInput")
    out_d = nc.dram_tensor("out", [B, TC, H], f32, kind="ExternalOutput")

    import contextlib

    with tile.TileContext(nc) as tc:
        loop_cm = tc.For_i(0, loop_n, 1) if loop_n is not None else contextlib.nullcontext()
        with (
            tc.tile_pool(name="const", bufs=1) as constp,
            tc.tile_pool(name="enc", bufs=3) as encp,
            tc.tile_pool(name="es", bufs=2) as esp,
            tc.tile_pool(name="x", bufs=2) as xp,
            tc.tile_pool(name="sm", bufs=3) as smp,
            tc.tile_pool(name="outs", bufs=3) as outp,
            tc.tile_pool(name="psA", bufs=2, space="PSUM") as psA,
            tc.tile_pool(name="psV", bufs=2, space="PSUM") as psV,
            tc.tile_pool(name="psC", bufs=1, space="PSUM") as psC,
            tc.tile_pool(name="psO", bufs=1, space="PSUM") as psO,
            loop_cm,
        ):
            # ---- persistent weights/activations ----
            # DMA order matters for pipeline fill: projQ deps (qt, wst) and
            # projE deps (wht) first; v/wot are not needed until the first
            # tail.
            qt_sb = constp.tile([128, KC, B, TC], bf16)
            nc.sync.dma_start(qt_sb[:], qt_d[:])
            wst = constp.tile([128, KC, H], bf16)
            nc.sync.dma_start(wst[:], wst_d[:])
            wht = constp.tile([128, KC, H], bf16)
            nc.sync.dma_start(wht[:], wht_d[:])
            v_sb = constp.tile([128, KC], bf16)
            nc.sync.dma_start(v_sb[:], v_d[:])
            wot = constp.tile([128, 2 * KC, H], bf16)
            nc.sync.dma_start(wot[:], wot_d[:])

            # ---- Ws_q^T for all (b, t): qs[o-part, oc, b, t] (fp32 scalars) ----
            qs_sb = constp.tile([128, OC, B, TC], f32)
            for oc in range(OC):
                ps = psA.tile([128, B * TC], f32, tag="proj")
                for kc in range(KC):
                    nc.tensor.matmul(
                        ps[:],
                        wst[:, kc, oc * 128 : (oc + 1) * 128],
                        qt_sb[:, kc, :, :],
                        start=(kc == 0),
                        stop=(kc == KC - 1),
                    )
                nc.vector.tensor_copy(
                    qs_sb[:, oc, :, :], ps.rearrange("p (b t) -> p b t", b=B)
                )

            # Software-pipelined emission: engines execute their streams in
            # order, so the tail of batch b (vdot/softmax/ctx/out — gated on
            # long dependency chains) is emitted AFTER the head of batch b+1
            # (proj/adds/tanh). This keeps DVE/ACT streaming without stalls.
            state = {}

            def head(b):
                L = Ls[b]
                # load encoder (both layouts), full-S tiles for clean DMA
                encT_b = encp.tile([128, KC, S], bf16, tag="encT")
                nc.sync.dma_start(encT_b[:], encT_d[b])
                enc_b = encp.tile([128, S // 128, H], bf16, tag="enc")
                nc.sync.dma_start(enc_b[:], enc_d[b])

                # Wh_e^T: es[o-part, s] per oc
                es = []
                for oc in range(OC):
                    ps = psA.tile([128, L], f32, tag="proj")
                    for kc in range(KC):
                        nc.tensor.matmul(
                            ps[:],
                            wht[:, kc, oc * 128 : (oc + 1) * 128],
                            encT_b[:, kc, :L],
                            start=(kc == 0),
                            stop=(kc == KC - 1),
                        )
                    e = esp.tile([128, L], bf16, tag=f"es{oc}")
                    nc.vector.tensor_copy(e[:], ps[:])
                    es.append(e)

                # X[o, t, s] = es[o, s] + qs[o, t]; tanh in place
                X = []
                for oc in range(OC):
                    x = xp.tile([128, TC, L], bf16, tag=f"x{oc}")
                    for t in range(TC if stages != 5 else 1):
                        nc.vector.tensor_scalar_add(
                            x[:, t, :], es[oc][:], qs_sb[:, oc, b, t : t + 1]
                        )
                    if stages not in (4, 5):
                        nc.scalar.activation(x[:], x[:], AF.Tanh)
                    X.append(x)
                state[b] = (X, enc_b)

            def tail(b):
                L = Ls[b]
                ln = min(int(lens[b]), S)
                SC = (L + 127) // 128
                L128 = SC * 128
                X, enc_b = state.pop(b)
                if stages == 1:
                    ob = outp.tile([32, 16], f32, tag="ob1")
                    nc.vector.tensor_copy(ob[:], X[0][:32, 0, :16])
                    nc.sync.dma_start(out_d[b][:, :16], ob[:])
                    return

                # energies[t, s] = sum_o v_o X[o, t, s]: M=1 matmuls, 16 t's
                # per PSUM tile (4 col groups x 4 bank slots), wide DVE evac,
                # partition->partition DMA gather. Note: accumulation groups
                # sharing a (partition, bank) zero-region must not interleave
                # (start=True marks the whole 2KB bank-row pending-zero);
                # col groups (distinct partitions) may interleave freely.
                energ = smp.tile([32, L], f32, tag="energ")
                for h in range(TC // 16):
                    psq = psV.tile([128, 4, 512 // 2], f32, tag="vdot")
                    for n in range(4):
                        for oc in range(OC):
                            for j in range(4):
                                t = h * 16 + 4 * n + j
                                nc.tensor.matmul(
                                    psq[32 * j : 32 * j + 1, n, :L],
                                    v_sb[:, oc : oc + 1],
                                    X[oc][:, t, :],
                                    start=(oc == 0),
                                    stop=(oc == OC - 1),
                                    tile_position=(0, 32 * j),
                                )
                    vscr = smp.tile([128, 4, L], f32, tag="vscr")
                    nc.vector.tensor_copy(vscr[:], psq[:, :, :L])
                    vsr = vscr.rearrange("(g r) n f -> g r n f", r=32)
                    for n in range(4):
                        nc.sync.dma_start(
                            energ[h * 16 + 4 * n : h * 16 + 4 * n + 4, :],
                            vsr[:, 0, n, :],
                        )

                if stages == 2:
                    ob = outp.tile([32, 16], f32, tag="ob1")
                    nc.vector.tensor_copy(ob[:], energ[:, :16])
                    nc.sync.dma_start(out_d[b][:, :16], ob[:])
                    return
                # softmax over s (energies bounded by sum|v| ~ 20: raw exp
                # is safe in fp32/bf16 — skip max-subtract)
                if L > ln:
                    nc.vector.memset(energ[:, ln:L], -1e9)
                w_sb = smp.tile([32, L128], bf16, tag="w")
                if L128 > L:
                    nc.vector.memset(w_sb[:, L:], 0.0)
                sm = smp.tile([32, 1], f32, tag="sm")
                nc.scalar.activation(
                    w_sb[:, :L], energ[:], AF.Exp, accum_out=sm[:]
                )
                rs = smp.tile([32, 1], f32, tag="rs")
                nc.vector.reciprocal(rs[:], sm[:])
                nc.vector.tensor_scalar_mul(w_sb[:, :L], w_sb[:, :L], rs[:])

                # w^T via DMA xbar transpose: [32, L128] -> [L128, 32]
                wT = smp.tile([128, SC, TC], bf16, tag="wT")
                for sc in range(SC):
                    nc.sync.dma_start_transpose(
                        wT[:, sc, :], w_sb[:, sc * 128 : (sc + 1) * 128]
                    )

                # ctx^T[h, t] = sum_s enc[s, h] * w[t, s]; all 4 oc slots in
                # one PSUM bank-row (groups are sequential per slot — legal)
                psc = psC.tile([128, OC, TC], f32, tag="ctx")
                for oc in range(OC):
                    for sc in range(SC):
                        nc.tensor.matmul(
                            psc[:, oc, :],
                            enc_b[:, sc, oc * 128 : (oc + 1) * 128],
                            wT[:, sc, :],
                            start=(sc == 0),
                            stop=(sc == SC - 1),
                        )
                ctxT = outp.tile([128, OC, TC], bf16, tag="ctxT")
                nc.vector.tensor_copy(ctxT[:], psc[:])

                # out[t, o] = tanh(sum_k comb^T[k, t] * W_out[o, k])
                pso = psO.tile([32, H], f32, tag="outp")
                for kc in range(2 * KC):
                    lhsT = (
                        ctxT[:, kc, :]
                        if kc < OC
                        else qt_sb[:, kc - OC, b, :]
                    )
                    nc.tensor.matmul(
                        pso[:],
                        lhsT,
                        wot[:, kc, :],
                        start=(kc == 0),
                        stop=(kc == 2 * KC - 1),
                    )
                ob = outp.tile([32, H], f32, tag="ob")
                nc.scalar.activation(ob[:], pso[:], AF.Tanh)
                nc.sync.dma_start(out_d[b], ob[:])

            # Descending-L order: the pipeline tail drain (last batch's
            # tail with no head to overlap) is paid on the smallest batch.
            order = sorted(range(B), key=lambda b: -Ls[b])
            for i, b in enumerate(order):
                head(b)
                if i > 0:
                    tail(order[i - 1])
            tail(order[-1])

    nc.compile()
    return nc


def _prep_inputs(query, encoder_outputs, src_lengths, W_s, W_h, v, W_out):
    """Host-side: cast to bf16 and pre-arrange into SBUF layouts."""
    q = np.asarray(query, np.float32)
    e = np.asarray(encoder_outputs, np.float32)

    # [128, KC, B, TC] per core: qt[p, kc, b, t] = q[b, c*TC+t, kc*128+p]
    # build once for full T then slice per core.
    qt_full = np.transpose(
        q.reshape(B, T, KC, 128), (2, 3, 0, 1)
    )  # [KC, 128, B, T]
    qt_full = np.ascontiguousarray(np.swapaxes(qt_full, 0, 1)).astype(_BF16)
    # -> [128, KC, B, T]

    # encT[b, p, kc, s] = e[b, s, kc*128+p]
    encT = np.ascontiguousarray(
        np.transpose(e.reshape(B, S, KC, 128), (0, 3, 2, 1))
    ).astype(_BF16)
    # enc[b, p, sc, h] = e[b, sc*128+p, h]
    enc = np.ascontiguousarray(
        np.transpose(e.reshape(B, S // 128, 128, H), (0, 2, 1, 3))
    ).astype(_BF16)

    # wst[p, kc, o] = W_s[o, kc*128+p]
    wst = np.ascontiguousarray(
        np.transpose(np.asarray(W_s, np.float32).reshape(H, KC, 128), (2, 1, 0))
    ).astype(_BF16)
    wht = np.ascontiguousarray(
        np.transpose(np.asarray(W_h, np.float32).reshape(H, KC, 128), (2, 1, 0))
    ).astype(_BF16)
    # v[p, kc] = v[kc*128+p]
    v_pre = np.ascontiguousarray(
        np.asarray(v, np.float32).reshape(KC, 128).T
    ).astype(_BF16)
    # wot[p, kc, o] = W_out[o, kc*128+p]   (k = 2H contraction)
    wot = np.ascontiguousarray(
        np.transpose(np.asarray(W_out, np.float32).reshape(H, 2 * KC, 128), (2, 1, 0))
    ).astype(_BF16)

    lens = tuple(int(x) for x in np.asarray(src_lengths).reshape(-1))
    return qt_full, encT, enc, wst, wht, v_pre, wot, lens


def kernel(query, encoder_outputs, src_lengths, W_s, W_h, v, W_out):
    from concourse import bass_utils

    qt_full, encT, enc, wst, wht, v_pre, wot, lens = _prep_inputs(
        query, encoder_outputs, src_lengths, W_s, W_h, v, W_out
    )
    nc = _build(lens)

    in_maps = []
    for c in range(NCORES):
        qt_c = np.ascontiguousarray(
            qt_full[:, :, :, c * TC : (c + 1) * TC]
        )
        in_maps.append(
            {
                "qt": qt_c,
                "encT": encT,
                "enc": enc,
                "wst": wst,
                "wht": wht,
                "v": v_pre,
                "wot": wot,
            }
        )

    res = bass_utils.run_bass_kernel_spmd(nc, in_maps, core_ids=list(range(NCORES)))

    out = np.empty((B, T, H), np.float32)
    for c in range(NCORES):
        out[:, c * TC : (c + 1) * TC, :] = res.results[c]["out"]
    return out
